# revision 19
# baseline (speedup 1.0000x reference)
"""Trainium2 Bass kernel for nn_Attention_30554397344218.

Multi-head attention (B=8, S=1040, D=1024, H=16, hd=64) with 2D vision RoPE
on the 1024 grid tokens after a 16-token puzzle prefix.

Strategy: pure data-parallel — one batch element per NeuronCore (8 cores,
no collectives). Per core, everything is computed in bf16 on the
TensorEngine with f32 PSUM accumulation:

  - host passes x pre-transposed (xT: D x S) plus flattened weights, 2D-RoPE
    cos/sin tables (identity prefix for the puzzle tokens) and a 128x128
    partition-swap permutation matrix
  - QKV projections produce q,k in transposed layout (head_dim on the
    partition axis, two heads stacked per 128-partition chunk) and v in
    natural layout with an appended ones-column per head
  - RoPE: swap halves via a PE permutation matmul, then q*cos + swap(q)*sin
    on the VectorEngine
  - scores are computed transposed (keys on partitions), exp(S/8) runs on
    the ScalarEngine straight out of PSUM (no max-subtraction needed:
    scores are ~N(0,1)), the ones-column makes attn@v also emit the softmax
    denominator as a 65th output row
  - normalization happens on the O^T tiles (reciprocal + partition
    broadcast), fused into the PSUM->SBUF cast
  - output projection accumulates the 8 head-pair chunks into f32 and DMAs
    out
"""

import numpy as np
import ml_dtypes

B, S, D, H, HD = 8, 1040, 1024, 16, 64
PFX = 16  # puzzle prefix length
GRID = 32
NCHUNK = 8  # 128-row chunks of the (1024,) head-dim axis
NJT = 9  # 128-row tiles of the 1040 seq axis (last tile = 16 rows)
ICH = [(0, 512), (512, 512), (1024, 16)]  # free-dim chunks of the seq axis
BF16 = ml_dtypes.bfloat16

_compiled = None  # cached (nc, const_in_map)


def _rope_tables():
    half, quarter = HD // 2, HD // 4
    frac = 2.0 * np.arange(quarter, dtype=np.float64) / half
    ts = 10000.0 ** frac
    row = np.arange(GRID, dtype=np.float64)[:, None] / ts[None, :]  # (32, 16)
    row_ang = np.broadcast_to(row[:, None, :], (GRID, GRID, quarter)).reshape(
        GRID * GRID, quarter
    )
    col_ang = np.broadcast_to(row[None, :, :], (GRID, GRID, quarter)).reshape(
        GRID * GRID, quarter
    )
    cos64 = np.concatenate(
        [np.cos(row_ang).T, np.cos(row_ang).T, np.cos(col_ang).T, np.cos(col_ang).T],
        axis=0,
    )  # (64, 1024)
    s64 = np.concatenate(
        [-np.sin(row_ang).T, np.sin(row_ang).T, -np.sin(col_ang).T, np.sin(col_ang).T],
        axis=0,
    )
    cosf = np.ones((HD, S), np.float64)
    sf = np.zeros((HD, S), np.float64)
    cosf[:, PFX:] = cos64
    sf[:, PFX:] = s64
    cos2 = np.concatenate([cosf, cosf], axis=0).astype(BF16)  # (128, 1040)
    s2 = np.concatenate([sf, sf], axis=0).astype(BF16)
    return cos2, s2


def _swap_matrix():
    swp = np.zeros((128, 128), np.float32)
    for i in range(128):
        swp[i, i ^ 16] = 1.0
    return swp.astype(BF16)


def _build_body(nc, tc, tile, mybir, aps):
    from contextlib import ExitStack

    bf = mybir.dt.bfloat16
    f32 = mybir.dt.float32
    Exp = mybir.ActivationFunctionType.Exp
    xT, Wq, Wk, Wv, Wo, COS2, S2, SWP, OUT = (
        aps["xT"], aps["Wq"], aps["Wk"], aps["Wv"], aps["Wo"],
        aps["COS2"], aps["S2"], aps["SWP"], aps["out"],
    )
    TAIL = S - 8 * 128  # 16

    with ExitStack() as ctx:
        # ---- persistent pools (live for the whole kernel)
        p_qk = ctx.enter_context(tc.tile_pool(name="qk", bufs=24))
        p_vx = ctx.enter_context(tc.tile_pool(name="vx", bufs=9))
        p_ot = ctx.enter_context(tc.tile_pool(name="ot", bufs=8))
        p_tab = ctx.enter_context(tc.tile_pool(name="tab", bufs=1))

        cos_sb = p_tab.tile([128, S], bf, tag="cos")
        s_sb = p_tab.tile([128, S], bf, tag="sin")
        swp_sb = p_tab.tile([128, 128], bf, tag="swp")
        ones64 = p_tab.tile([64, S], f32, tag="ones64")
        nc.vector.memset(ones64, 1.0)
        sel16 = p_tab.tile([2, 128], f32, tag="sel16")
        nc.sync.dma_start(out=sel16, in_=aps["SEL"][:, :])
        nc.sync.dma_start(out=cos_sb, in_=COS2[:, :])
        nc.sync.dma_start(out=s_sb, in_=S2[:, :])
        nc.sync.dma_start(out=swp_sb, in_=SWP[:, :])

        # q chunks (2 heads each); k chunks duplicated with one head zeroed so
        # the score matmuls contract a full K=128 (fast PE path): krz[c][0]
        # keeps rows 0:64 (even head), krz[c][1] keeps rows 64:128 (odd head).
        p_wo = ctx.enter_context(tc.tile_pool(name="wo", bufs=8))
        wo_t = []
        for k in range(8):
            t = p_wo.tile([128, D], bf, tag="wo", name=f"wo{k}")
            nc.sync.dma_start(out=t, in_=Wo[k * 128 : (k + 1) * 128, :])
            wo_t.append(t)

        qr = [p_qk.tile([128, S], bf, tag="qk", name=f"qr{i}") for i in range(NCHUNK)]
        krz = [
            [p_qk.tile([128, S], bf, tag="qk", name=f"krz{i}_{z}") for z in range(2)]
            for i in range(NCHUNK)
        ]
        # v tiles padded to M=128 per head: [v_h (64) | ones | zeros (63)]
        vx = [
            p_vx.tile([128, 16, 128], bf, tag="vx", name=f"vx{i}") for i in range(NJT)
        ]
        otc = [p_ot.tile([128, S], bf, tag="ot", name=f"otc{i}") for i in range(NCHUNK)]

        # ================= phase 1: projections + RoPE =================
        with ExitStack() as p1:
            p_x = p1.enter_context(tc.tile_pool(name="x", bufs=8))
            p_w = p1.enter_context(tc.tile_pool(name="w", bufs=8))
            p_tmp = p1.enter_context(tc.tile_pool(name="tmp", bufs=3))
            p_ps1 = p1.enter_context(tc.tile_pool(name="ps1", bufs=6, space="PSUM"))
            p_ps2 = p1.enter_context(tc.tile_pool(name="ps2", bufs=2, space="PSUM"))

            xt = [p_x.tile([128, S], bf, tag="xt", name=f"xt{k}") for k in range(8)]
            wq_t = [p_w.tile([128, D], bf, tag="wq", name=f"wq{k}") for k in range(8)]
            wk_t = [p_w.tile([128, D], bf, tag="wk", name=f"wk{k}") for k in range(8)]
            wv_t = [p_w.tile([128, D], bf, tag="wv", name=f"wv{k}") for k in range(8)]
            half = S // 2
            for k in range(8):
                src_x = xT[k * 128 : (k + 1) * 128, :]
                nc.sync.dma_start(out=xt[k][:, :half], in_=src_x[:, :half])
                nc.sync.dma_start(out=xt[k][:, half:], in_=src_x[:, half:])
                src_w = Wq[k * 128 : (k + 1) * 128, :]
                nc.sync.dma_start(out=wq_t[k][:, :512], in_=src_w[:, :512])
                nc.sync.dma_start(out=wq_t[k][:, 512:], in_=src_w[:, 512:])
            for W, lst in ((Wk, wk_t), (Wv, wv_t)):
                for k in range(8):
                    nc.sync.dma_start(out=lst[k], in_=W[k * 128 : (k + 1) * 128, :])

            # zero-fill the pad regions once
            for c in range(NCHUNK):
                nc.vector.memset(krz[c][0][64:128, :], 0.0)
                nc.vector.memset(krz[c][1][0:64, :], 0.0)
            for j in range(NJT):
                nc.vector.memset(vx[j][:, :, 64:128], 0.0)

            # --- q/k in transposed layout + RoPE (k-outer for lhsT reuse)
            for which, w_t in (("q", wq_t), ("k", wk_t)):
                for c in range(NCHUNK):
                    raw = p_tmp.tile([128, S], bf, tag="raw")
                    pss = [
                        p_ps1.tile([128, 512], f32, tag="mm1", name=f"mm1_{which}{c}_{i}")
                        for i in range(3)
                    ]
                    for k in range(8):
                        for i, (off, wdt) in enumerate(ICH):
                            nc.tensor.matmul(
                                pss[i][:, :wdt],
                                w_t[k][:, c * 128 : (c + 1) * 128],
                                xt[k][:, off : off + wdt],
                                start=(k == 0),
                                stop=(k == 7),
                            )
                    for i, (off, wdt) in enumerate(ICH):
                        nc.vector.tensor_copy(raw[:, off : off + wdt], pss[i][:, :wdt])
                    for off, wdt in ICH:
                        sw = p_ps2.tile([128, 512], f32, tag="swp")
                        nc.tensor.matmul(
                            sw[:, :wdt],
                            swp_sb,
                            raw[:, off : off + wdt],
                            start=True,
                            stop=True,
                        )
                        t1 = p_tmp.tile([128, 512], bf, tag="t1")
                        nc.vector.tensor_mul(
                            t1[:, :wdt],
                            raw[:, off : off + wdt],
                            cos_sb[:, off : off + wdt],
                        )
                        t2 = p_tmp.tile([128, 512], bf, tag="t2")
                        nc.vector.tensor_mul(
                            t2[:, :wdt], sw[:, :wdt], s_sb[:, off : off + wdt]
                        )
                        if which == "q":
                            nc.vector.tensor_add(
                                qr[c][:, off : off + wdt], t1[:, :wdt], t2[:, :wdt]
                            )
                        else:
                            nc.vector.tensor_add(
                                krz[c][0][0:64, off : off + wdt],
                                t1[0:64, :wdt],
                                t2[0:64, :wdt],
                            )
                            nc.vector.tensor_add(
                                krz[c][1][64:128, off : off + wdt],
                                t1[64:128, :wdt],
                                t2[64:128, :wdt],
                            )

            # --- v in natural layout (j-tiles), ones col at 64, zeros above
            for j in range(NJT):
                rows = 128 if j < 8 else TAIL
                nc.vector.memset(vx[j][:rows, :, 64:65], 1.0)
                psv = [
                    p_ps1.tile([128, 512], f32, tag="mm1", name=f"mm1_v{j}_{i}")
                    for i in range(2)
                ]
                for k in range(8):
                    for ci in range(2):
                        nc.tensor.matmul(
                            psv[ci][:rows, :],
                            xt[k][:, j * 128 : j * 128 + rows],
                            wv_t[k][:, ci * 512 : (ci + 1) * 512],
                            start=(k == 0),
                            stop=(k == 7),
                        )
                for ci in range(2):
                    nc.vector.tensor_copy(
                        vx[j][:rows, ci * 8 : (ci + 1) * 8, 0:64],
                        psv[ci][:rows, :].rearrange("p (h d) -> p h d", h=8),
                    )

        # ================= phase 2: attention per head =================
        with ExitStack() as p2:
            p_pt = p2.enter_context(tc.tile_pool(name="pt", bufs=27))
            p_sm = p2.enter_context(tc.tile_pool(name="sm", bufs=1))
            p_st = p2.enter_context(tc.tile_pool(name="st", bufs=2, space="PSUM"))
            p_po = p2.enter_context(tc.tile_pool(name="po", bufs=2, space="PSUM"))

            cs_cur = [None]
            for h in range(H):
                c, hb = divmod(h, 2)
                if hb == 0:
                    cs_cur[0] = p_sm.tile([2, S], f32, tag="cs", name=f"cs{c}", bufs=3)
                cstage = p_sm.tile([65, S], f32, tag="cstage", name=f"cst{h}", bufs=2)
                pts = []
                for j in range(NJT):
                    rows = 128 if j < 8 else TAIL
                    pt = p_pt.tile([128, S], bf, tag="pt")
                    pts.append(pt)
                    st = p_st.tile([128, S], f32, tag="st")
                    for off, wdt in ICH:
                        nc.tensor.matmul(
                            st[:rows, off : off + wdt],
                            krz[c][hb][:, j * 128 : j * 128 + rows],
                            qr[c][:, off : off + wdt],
                            start=True,
                            stop=True,
                        )
                    nc.scalar.activation(
                        pt[:rows, :],
                        st[:rows, :],
                        Exp,
                        scale=1.0 / np.sqrt(HD),
                    )
                for off, wdt in ICH:
                    ot = p_po.tile([128, 512], f32, tag="ot")
                    for j in range(NJT):
                        rows = 128 if j < 8 else TAIL
                        nc.tensor.matmul(
                            ot[:, :wdt],
                            vx[j][:rows, h, :],
                            pts[j][:rows, off : off + wdt],
                            start=(j == 0),
                            stop=(j == NJT - 1),
                        )
                    nc.vector.tensor_mul(
                        otc[c][hb * 64 : hb * 64 + 64, off : off + wdt],
                        ot[0:64, :wdt],
                        ones64[:, off : off + wdt],
                    )
                    nc.vector.tensor_copy(
                        cstage[64:65, off : off + wdt], ot[64:65, :wdt]
                    )

                nc.sync.dma_start(
                    out=cs_cur[0][hb : hb + 1, :], in_=cstage[64:65, :]
                )
                if hb == 1:
                    rcp = p_sm.tile([2, S], f32, tag="rcp", name=f"rcp{c}", bufs=1)
                    nc.vector.reciprocal(rcp, cs_cur[0])
                    for off, wdt in ICH:
                        psb = p_po.tile([128, 512], f32, tag="ot", name=f"psb{c}_{off}")
                        nc.tensor.matmul(
                            psb[:, :wdt],
                            sel16,
                            rcp[:, off : off + wdt],
                            start=True,
                            stop=True,
                        )
                        nc.vector.tensor_mul(
                            otc[c][:, off : off + wdt],
                            otc[c][:, off : off + wdt],
                            psb[:, :wdt],
                        )

        # ================= phase 3: output projection =================
        with ExitStack() as p3:
            p_y = p3.enter_context(tc.tile_pool(name="y", bufs=4))
            p_py = p3.enter_context(tc.tile_pool(name="py", bufs=4, space="PSUM"))

            for it in range(NJT):
                rows = 128 if it < 8 else TAIL
                yps = [
                    p_py.tile([128, 512], f32, tag="y", name=f"y{it}_{i}")
                    for i in range(2)
                ]
                for c in range(NCHUNK):
                    for ci in range(2):
                        nc.tensor.matmul(
                            yps[ci][:rows, :],
                            otc[c][:, it * 128 : it * 128 + rows],
                            wo_t[c][:, ci * 512 : (ci + 1) * 512],
                            start=(c == 0),
                            stop=(c == 7),
                        )
                for ci in range(2):
                    ysb = p_y.tile([128, 512], f32, tag="ysb")
                    nc.vector.tensor_copy(ysb[:rows, :], yps[ci][:rows, :])
                    nc.sync.dma_start(
                        out=OUT[it * 128 : it * 128 + rows, ci * 512 : (ci + 1) * 512],
                        in_=ysb[:rows, :],
                    )


def _build():
    global _compiled
    if _compiled is not None:
        return _compiled
    import concourse.bass as bass  # noqa: F401
    import concourse.mybir as mybir
    import concourse.tile as tile
    from concourse import bacc

    nc = bacc.Bacc("TRN2", target_bir_lowering=False, debug=False)
    bf = mybir.dt.bfloat16
    f32 = mybir.dt.float32
    aps = {
        "xT": nc.dram_tensor("xT", [D, S], bf, kind="ExternalInput").ap(),
        "Wq": nc.dram_tensor("Wq", [D, H * HD], bf, kind="ExternalInput").ap(),
        "Wk": nc.dram_tensor("Wk", [D, H * HD], bf, kind="ExternalInput").ap(),
        "Wv": nc.dram_tensor("Wv", [D, H * HD], bf, kind="ExternalInput").ap(),
        "Wo": nc.dram_tensor("Wo", [H * HD, D], bf, kind="ExternalInput").ap(),
        "COS2": nc.dram_tensor("COS2", [128, S], bf, kind="ExternalInput").ap(),
        "S2": nc.dram_tensor("S2", [128, S], bf, kind="ExternalInput").ap(),
        "SWP": nc.dram_tensor("SWP", [128, 128], bf, kind="ExternalInput").ap(),
        "SEL": nc.dram_tensor("SEL", [2, 128], f32, kind="ExternalInput").ap(),
        "out": nc.dram_tensor("out", [S, D], f32, kind="ExternalOutput").ap(),
    }
    with tile.TileContext(nc) as tc:
        _build_body(nc, tc, tile, mybir, aps)
    nc.compile()
    _compiled = nc
    return nc


def _install_trace_shim():
    """The agent image's antenv lacks axon_hooks, so run_bass_kernel_spmd's
    trace path can't find the NTFF profile hook trn_boot would have set.
    Recreate the module and install the ctypes hook; skip the S3 artifact
    upload (no creds needed for local timing)."""
    import sys
    import types

    if "antenv.axon_hooks" not in sys.modules:
        import antenv  # noqa: F401

        mod = types.ModuleType("antenv.axon_hooks")
        mod._hook = None

        def set_axon_ntff_profile_hook(h):
            mod._hook = h

        def get_axon_ntff_profile_hook():
            return mod._hook

        mod.set_axon_ntff_profile_hook = set_axon_ntff_profile_hook
        mod.get_axon_ntff_profile_hook = get_axon_ntff_profile_hook
        sys.modules["antenv.axon_hooks"] = mod

    import antenv.axon_hooks as ah

    if ah.get_axon_ntff_profile_hook() is None:
        from trn_agent_boot.trn_boot import _ntff_profile_via_ctypes

        ah.set_axon_ntff_profile_hook(
            _ntff_profile_via_ctypes("/opt/axon/libaxon_pjrt.so")
        )

    import concourse.bass_utils as bu

    bu.upload_artifacts = lambda tmpdir: f"local://{tmpdir}"


def run(inputs, trace=False):
    """Returns (output (8,1040,1024) f32, exec_time_ns or None)."""
    if trace:
        _install_trace_shim()
    from concourse.bass_utils import run_bass_kernel_spmd

    nc = _build()
    x = np.asarray(inputs["x"], np.float32)
    wq = np.asarray(inputs["Wq"], np.float32).reshape(D, H * HD).astype(BF16)
    wk = np.asarray(inputs["Wk"], np.float32).reshape(D, H * HD).astype(BF16)
    wv = np.asarray(inputs["Wv"], np.float32).reshape(D, H * HD).astype(BF16)
    wo = np.asarray(inputs["Wo"], np.float32).reshape(H * HD, D).astype(BF16)
    cos2, s2 = _rope_tables()
    swp = _swap_matrix()
    sel = np.zeros((2, 128), np.float32)
    sel[0, 0:64] = 1.0
    sel[1, 64:128] = 1.0
    shared = {
        "Wq": wq, "Wk": wk, "Wv": wv, "Wo": wo,
        "COS2": cos2, "S2": s2, "SWP": swp, "SEL": sel,
    }
    in_maps = [
        dict(shared, xT=np.ascontiguousarray(x[b].T).astype(BF16)) for b in range(B)
    ]
    res = run_bass_kernel_spmd(nc, in_maps, core_ids=list(range(B)), trace=trace)
    out = np.stack([np.asarray(r["out"], np.float32) for r in res.results], axis=0)
    return out, res.exec_time_ns


def kernel(x, Wq, Wk, Wv, Wo):
    out, _ = run({"x": x, "Wq": Wq, "Wk": Wk, "Wv": Wv, "Wo": Wo})
    return out


# revision 23
# speedup vs baseline: 1.1044x; 1.1044x over previous
"""Trainium2 Bass kernel for nn_Attention_30554397344218.

Multi-head attention (B=8, S=1040, D=1024, H=16, hd=64) with 2D vision RoPE
on the 1024 grid tokens after a 16-token puzzle prefix.

Strategy: pure data-parallel — one batch element per NeuronCore (8 cores,
no collectives). Per core, everything is computed in bf16 on the
TensorEngine with f32 PSUM accumulation:

  - host passes x pre-transposed (xT: D x S) plus flattened weights, 2D-RoPE
    cos/sin tables (identity prefix for the puzzle tokens) and a 128x128
    partition-swap permutation matrix
  - QKV projections produce q,k in transposed layout (head_dim on the
    partition axis, two heads stacked per 128-partition chunk) and v in
    natural layout with an appended ones-column per head
  - RoPE: swap halves via a PE permutation matmul, then q*cos + swap(q)*sin
    on the VectorEngine
  - scores are computed transposed (keys on partitions), exp(S/8) runs on
    the ScalarEngine straight out of PSUM (no max-subtraction needed:
    scores are ~N(0,1)), the ones-column makes attn@v also emit the softmax
    denominator as a 65th output row
  - normalization happens on the O^T tiles (reciprocal + partition
    broadcast), fused into the PSUM->SBUF cast
  - output projection accumulates the 8 head-pair chunks into f32 and DMAs
    out
"""

import numpy as np
import ml_dtypes

B, S, D, H, HD = 8, 1040, 1024, 16, 64
PFX = 16  # puzzle prefix length
GRID = 32
NCHUNK = 8  # 128-row chunks of the (1024,) head-dim axis
NJT = 9  # 128-row tiles of the 1040 seq axis (last tile = 16 rows)
ICH = [(0, 512), (512, 512), (1024, 16)]  # free-dim chunks of the seq axis
BF16 = ml_dtypes.bfloat16

_compiled = None  # cached (nc, const_in_map)


def _rope_tables():
    half, quarter = HD // 2, HD // 4
    frac = 2.0 * np.arange(quarter, dtype=np.float64) / half
    ts = 10000.0 ** frac
    row = np.arange(GRID, dtype=np.float64)[:, None] / ts[None, :]  # (32, 16)
    row_ang = np.broadcast_to(row[:, None, :], (GRID, GRID, quarter)).reshape(
        GRID * GRID, quarter
    )
    col_ang = np.broadcast_to(row[None, :, :], (GRID, GRID, quarter)).reshape(
        GRID * GRID, quarter
    )
    cos64 = np.concatenate(
        [np.cos(row_ang).T, np.cos(row_ang).T, np.cos(col_ang).T, np.cos(col_ang).T],
        axis=0,
    )  # (64, 1024)
    s64 = np.concatenate(
        [-np.sin(row_ang).T, np.sin(row_ang).T, -np.sin(col_ang).T, np.sin(col_ang).T],
        axis=0,
    )
    cosf = np.ones((HD, S), np.float64)
    sf = np.zeros((HD, S), np.float64)
    cosf[:, PFX:] = cos64
    sf[:, PFX:] = s64
    cos2 = np.concatenate([cosf, cosf], axis=0).astype(BF16)  # (128, 1040)
    s2 = np.concatenate([sf, sf], axis=0).astype(BF16)
    return cos2, s2


def _swap_matrix():
    swp = np.zeros((128, 128), np.float32)
    for i in range(128):
        swp[i, i ^ 16] = 1.0
    return swp.astype(BF16)


def _build_body(nc, tc, tile, mybir, aps):
    from contextlib import ExitStack

    bf = mybir.dt.bfloat16
    f32 = mybir.dt.float32
    Exp = mybir.ActivationFunctionType.Exp
    xT, Wq, Wk, Wv, Wo, COS2, S2, SWP, OUT = (
        aps["xT"], aps["Wq"], aps["Wk"], aps["Wv"], aps["Wo"],
        aps["COS2"], aps["S2"], aps["SWP"], aps["out"],
    )
    TAIL = S - 8 * 128  # 16

    with ExitStack() as ctx:
        # ---- persistent pools
        p_qk = ctx.enter_context(tc.tile_pool(name="qk", bufs=24))
        p_vx = ctx.enter_context(tc.tile_pool(name="vx", bufs=9))
        p_ot = ctx.enter_context(tc.tile_pool(name="ot", bufs=8))
        p_tab = ctx.enter_context(tc.tile_pool(name="tab", bufs=1))
        p_wo = ctx.enter_context(tc.tile_pool(name="wo", bufs=8))

        cos_sb = p_tab.tile([128, S], bf, tag="cos")
        s_sb = p_tab.tile([128, S], bf, tag="sin")
        swp_sb = p_tab.tile([128, 128], bf, tag="swp")
        sel16 = p_tab.tile([16, 1024], f32, tag="sel16")
        ones64 = p_tab.tile([64, S], f32, tag="ones64")
        nc.vector.memset(ones64, 1.0)

        qr = [p_qk.tile([128, S], bf, tag="qk", name=f"qr{i}") for i in range(NCHUNK)]
        krz = [
            [p_qk.tile([128, S], bf, tag="qk", name=f"krz{i}_{z}") for z in range(2)]
            for i in range(NCHUNK)
        ]
        vx = [
            p_vx.tile([128, 1104], bf, tag="vx", name=f"vx{i}") for i in range(NJT)
        ]
        otc = [p_ot.tile([128, S], bf, tag="ot", name=f"otc{i}") for i in range(NCHUNK)]
        wo_t = [p_wo.tile([128, D], bf, tag="wo", name=f"wo{k}") for k in range(8)]

        # ================= phase 1: projections + RoPE =================
        with ExitStack() as p1:
            p_x = p1.enter_context(tc.tile_pool(name="x", bufs=8))
            p_w = p1.enter_context(tc.tile_pool(name="w", bufs=8))
            p_tmp = p1.enter_context(tc.tile_pool(name="tmp", bufs=3))
            p_ps1 = p1.enter_context(tc.tile_pool(name="ps1", bufs=6, space="PSUM"))
            p_ps2 = p1.enter_context(tc.tile_pool(name="ps2", bufs=2, space="PSUM"))

            xt = [p_x.tile([128, S], bf, tag="xt", name=f"xt{k}") for k in range(8)]
            wq_t = [p_w.tile([128, D], bf, tag="wq", name=f"wq{k}") for k in range(8)]
            wk_t = [p_w.tile([128, D], bf, tag="wk", name=f"wk{k}") for k in range(8)]
            wv_t = [p_w.tile([128, D], bf, tag="wv", name=f"wv{k}") for k in range(8)]
            q4 = S // 4
            for k in range(8):
                src_x = xT[k * 128 : (k + 1) * 128, :]
                for qq in range(4):
                    nc.sync.dma_start(
                        out=xt[k][:, qq * q4 : (qq + 1) * q4],
                        in_=src_x[:, qq * q4 : (qq + 1) * q4],
                    )
                src_w = Wq[k * 128 : (k + 1) * 128, :]
                for qq in range(4):
                    nc.sync.dma_start(
                        out=wq_t[k][:, qq * 256 : (qq + 1) * 256],
                        in_=src_w[:, qq * 256 : (qq + 1) * 256],
                    )
            nc.sync.dma_start(out=swp_sb, in_=SWP[:, :])
            nc.sync.dma_start(out=cos_sb, in_=COS2[:, :])
            nc.sync.dma_start(out=s_sb, in_=S2[:, :])
            for k in range(8):
                nc.sync.dma_start(out=wk_t[k], in_=Wk[k * 128 : (k + 1) * 128, :])
            for k in range(8):
                nc.sync.dma_start(out=wv_t[k], in_=Wv[k * 128 : (k + 1) * 128, :])
            for k in range(8):
                nc.sync.dma_start(out=wo_t[k], in_=Wo[k * 128 : (k + 1) * 128, :])
            nc.sync.dma_start(out=sel16, in_=aps["SEL"][:, :])

            for c in range(NCHUNK):
                nc.vector.memset(krz[c][0][64:128, :], 0.0)
                nc.vector.memset(krz[c][1][0:64, :], 0.0)

            # q/k in transposed layout + RoPE (k-outer for lhsT reuse)
            for which, w_t in (("q", wq_t), ("k", wk_t)):
                for c in range(NCHUNK):
                    raw = p_tmp.tile([128, S], bf, tag="raw")
                    pss = [
                        p_ps1.tile(
                            [128, 512], f32, tag="mm1", name=f"mm1_{which}{c}_{i}"
                        )
                        for i in range(3)
                    ]
                    for k in range(8):
                        for i, (off, wdt) in enumerate(ICH):
                            nc.tensor.matmul(
                                pss[i][:, :wdt],
                                w_t[k][:, c * 128 : (c + 1) * 128],
                                xt[k][:, off : off + wdt],
                                start=(k == 0),
                                stop=(k == 7),
                            )
                    for i, (off, wdt) in enumerate(ICH):
                        nc.vector.tensor_copy(raw[:, off : off + wdt], pss[i][:, :wdt])
                    for off, wdt in ICH:
                        sw = p_ps2.tile([128, 512], f32, tag="swp")
                        nc.tensor.matmul(
                            sw[:, :wdt],
                            swp_sb,
                            raw[:, off : off + wdt],
                            start=True,
                            stop=True,
                        )
                        t1 = p_tmp.tile([128, 512], bf, tag="t1")
                        nc.vector.tensor_mul(
                            t1[:, :wdt],
                            raw[:, off : off + wdt],
                            cos_sb[:, off : off + wdt],
                        )
                        t2 = p_tmp.tile([128, 512], bf, tag="t2")
                        nc.vector.tensor_mul(
                            t2[:, :wdt], sw[:, :wdt], s_sb[:, off : off + wdt]
                        )
                        if which == "q":
                            nc.vector.tensor_add(
                                qr[c][:, off : off + wdt], t1[:, :wdt], t2[:, :wdt]
                            )
                        else:
                            nc.vector.tensor_add(
                                krz[c][0][0:64, off : off + wdt],
                                t1[0:64, :wdt],
                                t2[0:64, :wdt],
                            )
                            nc.vector.tensor_add(
                                krz[c][1][64:128, off : off + wdt],
                                t1[64:128, :wdt],
                                t2[64:128, :wdt],
                            )

            # v in natural layout, overlapped [v_h | 1] blocks (stride 65)
            for j in range(NJT):
                rows = 128 if j < 8 else TAIL
                vx3 = vx[j][:, :1040].rearrange("p (h d) -> p h d", d=65)
                nc.vector.memset(vx[j][:, 1040:1104], 0.0)
                nc.vector.memset(vx3[:rows, :, 64:65], 1.0)
                psv = [
                    p_ps1.tile([128, 512], f32, tag="mm1", name=f"mm1_v{j}_{i}")
                    for i in range(2)
                ]
                for k in range(8):
                    for ci in range(2):
                        nc.tensor.matmul(
                            psv[ci][:rows, :],
                            xt[k][:, j * 128 : j * 128 + rows],
                            wv_t[k][:, ci * 512 : (ci + 1) * 512],
                            start=(k == 0),
                            stop=(k == 7),
                        )
                for ci in range(2):
                    nc.vector.tensor_copy(
                        vx3[:rows, ci * 8 : (ci + 1) * 8, 0:64],
                        psv[ci][:rows, :].rearrange("p (h d) -> p h d", h=8),
                    )

        # ================= phase 2: attention per head =================
        with ExitStack() as p2:
            p_pt = p2.enter_context(tc.tile_pool(name="pt", bufs=27))
            p_sm = p2.enter_context(tc.tile_pool(name="sm", bufs=1))
            p_st = p2.enter_context(tc.tile_pool(name="st", bufs=2, space="PSUM"))
            p_po = p2.enter_context(tc.tile_pool(name="po", bufs=2, space="PSUM"))

            cs_all = p_sm.tile([16, S], f32, tag="cs")
            for h in range(H):
                c, hb = divmod(h, 2)
                pts = []
                for j in range(NJT):
                    rows = 128 if j < 8 else TAIL
                    pt = p_pt.tile([128, S], bf, tag="pt", name=f"pt{h}_{j}")
                    pts.append(pt)
                    st = p_st.tile([128, S], f32, tag="st", name=f"st{h}_{j}")
                    for off, wdt in ICH:
                        nc.tensor.matmul(
                            st[:rows, off : off + wdt],
                            krz[c][hb][:, j * 128 : j * 128 + rows],
                            qr[c][:, off : off + wdt],
                            start=True,
                            stop=True,
                        )
                    nc.scalar.activation(
                        pt[:rows, :], st[:rows, :], Exp, scale=1.0 / np.sqrt(HD)
                    )
                cstage = p_sm.tile([65, S], f32, tag="cstage", name=f"cst{h}", bufs=2)
                for off, wdt in ICH:
                    ot = p_po.tile([128, 512], f32, tag="ot", name=f"ot{h}_{off}")
                    for j in range(NJT):
                        rows = 128 if j < 8 else TAIL
                        nc.tensor.matmul(
                            ot[:, :wdt],
                            vx[j][:rows, h * 65 : h * 65 + 128],
                            pts[j][:rows, off : off + wdt],
                            start=(j == 0),
                            stop=(j == NJT - 1),
                        )
                    nc.vector.tensor_mul(
                        otc[c][hb * 64 : hb * 64 + 64, off : off + wdt],
                        ot[0:64, :wdt],
                        ones64[:, off : off + wdt],
                    )
                    nc.vector.tensor_copy(
                        cstage[64:65, off : off + wdt], ot[64:65, :wdt]
                    )
                nc.sync.dma_start(out=cs_all[h : h + 1, :], in_=cstage[64:65, :])

            rcp_all = p_sm.tile([16, S], f32, tag="rcp")
            nc.vector.reciprocal(rcp_all, cs_all)
            for c in range(NCHUNK):
                for off, wdt in ICH:
                    psb = p_po.tile([128, 512], f32, tag="ot", name=f"psb{c}_{off}")
                    nc.tensor.matmul(
                        psb[:, :wdt],
                        sel16[:, c * 128 : (c + 1) * 128],
                        rcp_all[:, off : off + wdt],
                        start=True,
                        stop=True,
                    )
                    nc.vector.tensor_mul(
                        otc[c][:, off : off + wdt],
                        otc[c][:, off : off + wdt],
                        psb[:, :wdt],
                    )

        # ================= phase 3: output projection =================
        with ExitStack() as p3:
            p_y = p3.enter_context(tc.tile_pool(name="y", bufs=4))
            p_py = p3.enter_context(tc.tile_pool(name="py", bufs=4, space="PSUM"))

            for it in range(NJT):
                rows = 128 if it < 8 else TAIL
                yps = [
                    p_py.tile([128, 512], f32, tag="y", name=f"y{it}_{i}")
                    for i in range(2)
                ]
                for c in range(NCHUNK):
                    for ci in range(2):
                        nc.tensor.matmul(
                            yps[ci][:rows, :],
                            otc[c][:, it * 128 : it * 128 + rows],
                            wo_t[c][:, ci * 512 : (ci + 1) * 512],
                            start=(c == 0),
                            stop=(c == 7),
                        )
                for ci in range(2):
                    ysb = p_y.tile([128, 512], f32, tag="ysb")
                    nc.vector.tensor_copy(ysb[:rows, :], yps[ci][:rows, :])
                    nc.sync.dma_start(
                        out=OUT[it * 128 : it * 128 + rows, ci * 512 : (ci + 1) * 512],
                        in_=ysb[:rows, :],
                    )


def _build():
    global _compiled
    if _compiled is not None:
        return _compiled
    import concourse.bass as bass  # noqa: F401
    import concourse.mybir as mybir
    import concourse.tile as tile
    from concourse import bacc

    nc = bacc.Bacc("TRN2", target_bir_lowering=False, debug=False)
    bf = mybir.dt.bfloat16
    f32 = mybir.dt.float32
    aps = {
        "xT": nc.dram_tensor("xT", [D, S], bf, kind="ExternalInput").ap(),
        "Wq": nc.dram_tensor("Wq", [D, H * HD], bf, kind="ExternalInput").ap(),
        "Wk": nc.dram_tensor("Wk", [D, H * HD], bf, kind="ExternalInput").ap(),
        "Wv": nc.dram_tensor("Wv", [D, H * HD], bf, kind="ExternalInput").ap(),
        "Wo": nc.dram_tensor("Wo", [H * HD, D], bf, kind="ExternalInput").ap(),
        "COS2": nc.dram_tensor("COS2", [128, S], bf, kind="ExternalInput").ap(),
        "S2": nc.dram_tensor("S2", [128, S], bf, kind="ExternalInput").ap(),
        "SWP": nc.dram_tensor("SWP", [128, 128], bf, kind="ExternalInput").ap(),
        "SEL": nc.dram_tensor("SEL", [16, 1024], f32, kind="ExternalInput").ap(),
        "out": nc.dram_tensor("out", [S, D], f32, kind="ExternalOutput").ap(),
    }
    with tile.TileContext(nc) as tc:
        _build_body(nc, tc, tile, mybir, aps)
    nc.compile()
    _compiled = nc
    return nc


def _install_trace_shim():
    """The agent image's antenv lacks axon_hooks, so run_bass_kernel_spmd's
    trace path can't find the NTFF profile hook trn_boot would have set.
    Recreate the module and install the ctypes hook; skip the S3 artifact
    upload (no creds needed for local timing)."""
    import sys
    import types

    if "antenv.axon_hooks" not in sys.modules:
        import antenv  # noqa: F401

        mod = types.ModuleType("antenv.axon_hooks")
        mod._hook = None

        def set_axon_ntff_profile_hook(h):
            mod._hook = h

        def get_axon_ntff_profile_hook():
            return mod._hook

        mod.set_axon_ntff_profile_hook = set_axon_ntff_profile_hook
        mod.get_axon_ntff_profile_hook = get_axon_ntff_profile_hook
        sys.modules["antenv.axon_hooks"] = mod

    import antenv.axon_hooks as ah

    if ah.get_axon_ntff_profile_hook() is None:
        from trn_agent_boot.trn_boot import _ntff_profile_via_ctypes

        ah.set_axon_ntff_profile_hook(
            _ntff_profile_via_ctypes("/opt/axon/libaxon_pjrt.so")
        )

    import concourse.bass_utils as bu

    bu.upload_artifacts = lambda tmpdir: f"local://{tmpdir}"


def run(inputs, trace=False):
    """Returns (output (8,1040,1024) f32, exec_time_ns or None)."""
    if trace:
        _install_trace_shim()
    from concourse.bass_utils import run_bass_kernel_spmd

    nc = _build()
    x = np.asarray(inputs["x"], np.float32)
    wq = np.asarray(inputs["Wq"], np.float32).reshape(D, H * HD).astype(BF16)
    wk = np.asarray(inputs["Wk"], np.float32).reshape(D, H * HD).astype(BF16)
    wv = np.asarray(inputs["Wv"], np.float32).reshape(D, H * HD).astype(BF16)
    wo = np.asarray(inputs["Wo"], np.float32).reshape(H * HD, D).astype(BF16)
    cos2, s2 = _rope_tables()
    swp = _swap_matrix()
    sel = np.zeros((16, 1024), np.float32)
    for c in range(8):
        for hb in range(2):
            sel[2 * c + hb, c * 128 + hb * 64 : c * 128 + hb * 64 + 64] = 1.0
    shared = {
        "Wq": wq, "Wk": wk, "Wv": wv, "Wo": wo,
        "COS2": cos2, "S2": s2, "SWP": swp, "SEL": sel,
    }
    in_maps = [
        dict(shared, xT=np.ascontiguousarray(x[b].T).astype(BF16)) for b in range(B)
    ]
    res = run_bass_kernel_spmd(nc, in_maps, core_ids=list(range(B)), trace=trace)
    out = np.stack([np.asarray(r["out"], np.float32) for r in res.results], axis=0)
    return out, res.exec_time_ns


def kernel(x, Wq, Wk, Wv, Wo):
    out, _ = run({"x": x, "Wq": Wq, "Wk": Wk, "Wv": Wv, "Wo": Wo})
    return out


# revision 24
# speedup vs baseline: 1.1491x; 1.0405x over previous
"""Trainium2 Bass kernel for nn_Attention_30554397344218.

Multi-head attention (B=8, S=1040, D=1024, H=16, hd=64) with 2D vision RoPE
on the 1024 grid tokens after a 16-token puzzle prefix.

Strategy: pure data-parallel — one batch element per NeuronCore (8 cores,
no collectives). Per core, everything is computed in bf16 on the
TensorEngine with f32 PSUM accumulation:

  - host passes x pre-transposed (xT: D x S) plus flattened weights, 2D-RoPE
    cos/sin tables (identity prefix for the puzzle tokens) and a 128x128
    partition-swap permutation matrix
  - QKV projections produce q,k in transposed layout (head_dim on the
    partition axis, two heads stacked per 128-partition chunk) and v in
    natural layout with an appended ones-column per head
  - RoPE: swap halves via a PE permutation matmul, then q*cos + swap(q)*sin
    on the VectorEngine
  - scores are computed transposed (keys on partitions), exp(S/8) runs on
    the ScalarEngine straight out of PSUM (no max-subtraction needed:
    scores are ~N(0,1)), the ones-column makes attn@v also emit the softmax
    denominator as a 65th output row
  - normalization happens on the O^T tiles (reciprocal + partition
    broadcast), fused into the PSUM->SBUF cast
  - output projection accumulates the 8 head-pair chunks into f32 and DMAs
    out
"""

import numpy as np
import ml_dtypes

B, S, D, H, HD = 8, 1040, 1024, 16, 64
PFX = 16  # puzzle prefix length
GRID = 32
NCHUNK = 8  # 128-row chunks of the (1024,) head-dim axis
NJT = 9  # 128-row tiles of the 1040 seq axis (last tile = 16 rows)
ICH = [(0, 512), (512, 512), (1024, 16)]  # free-dim chunks of the seq axis
BF16 = ml_dtypes.bfloat16

_compiled = None  # cached (nc, const_in_map)


def _rope_tables():
    half, quarter = HD // 2, HD // 4
    frac = 2.0 * np.arange(quarter, dtype=np.float64) / half
    ts = 10000.0 ** frac
    row = np.arange(GRID, dtype=np.float64)[:, None] / ts[None, :]  # (32, 16)
    row_ang = np.broadcast_to(row[:, None, :], (GRID, GRID, quarter)).reshape(
        GRID * GRID, quarter
    )
    col_ang = np.broadcast_to(row[None, :, :], (GRID, GRID, quarter)).reshape(
        GRID * GRID, quarter
    )
    cos64 = np.concatenate(
        [np.cos(row_ang).T, np.cos(row_ang).T, np.cos(col_ang).T, np.cos(col_ang).T],
        axis=0,
    )  # (64, 1024)
    s64 = np.concatenate(
        [-np.sin(row_ang).T, np.sin(row_ang).T, -np.sin(col_ang).T, np.sin(col_ang).T],
        axis=0,
    )
    cosf = np.ones((HD, S), np.float64)
    sf = np.zeros((HD, S), np.float64)
    cosf[:, PFX:] = cos64
    sf[:, PFX:] = s64
    cos2 = np.concatenate([cosf, cosf], axis=0).astype(BF16)  # (128, 1040)
    s2 = np.concatenate([sf, sf], axis=0).astype(BF16)
    return cos2, s2


def _swap_matrix():
    swp = np.zeros((128, 128), np.float32)
    for i in range(128):
        swp[i, i ^ 16] = 1.0
    return swp.astype(BF16)


def _build_body(nc, tc, tile, mybir, aps):
    from contextlib import ExitStack

    bf = mybir.dt.bfloat16
    f32 = mybir.dt.float32
    Exp = mybir.ActivationFunctionType.Exp
    xT, Wq, Wk, Wv, Wo, COS2, S2, SWP, OUT = (
        aps["xT"], aps["Wq"], aps["Wk"], aps["Wv"], aps["Wo"],
        aps["COS2"], aps["S2"], aps["SWP"], aps["out"],
    )
    TAIL = S - 8 * 128  # 16

    with ExitStack() as ctx:
        # ---- persistent pools
        p_qk = ctx.enter_context(tc.tile_pool(name="qk", bufs=24))
        p_vx = ctx.enter_context(tc.tile_pool(name="vx", bufs=9))
        p_ot = ctx.enter_context(tc.tile_pool(name="ot", bufs=8))
        p_tab = ctx.enter_context(tc.tile_pool(name="tab", bufs=1))
        p_wo = ctx.enter_context(tc.tile_pool(name="wo", bufs=8))

        cos_sb = p_tab.tile([128, S], bf, tag="cos")
        s_sb = p_tab.tile([128, S], bf, tag="sin")
        swp_sb = p_tab.tile([128, 128], bf, tag="swp")
        sel16 = p_tab.tile([16, 1024], f32, tag="sel16")
        ones64 = p_tab.tile([64, S], f32, tag="ones64")
        nc.gpsimd.memset(ones64, 1.0)

        qr = [p_qk.tile([128, S], bf, tag="qk", name=f"qr{i}") for i in range(NCHUNK)]
        krz = [
            [p_qk.tile([128, S], bf, tag="qk", name=f"krz{i}_{z}") for z in range(2)]
            for i in range(NCHUNK)
        ]
        vx = [
            p_vx.tile([128, 1104], bf, tag="vx", name=f"vx{i}") for i in range(NJT)
        ]
        otc = [p_ot.tile([128, S], bf, tag="ot", name=f"otc{i}") for i in range(NCHUNK)]
        wo_t = [p_wo.tile([128, D], bf, tag="wo", name=f"wo{k}") for k in range(8)]

        # ================= phase 1: projections + RoPE =================
        with ExitStack() as p1:
            p_x = p1.enter_context(tc.tile_pool(name="x", bufs=8))
            p_w = p1.enter_context(tc.tile_pool(name="w", bufs=8))
            p_tmp = p1.enter_context(tc.tile_pool(name="tmp", bufs=3))
            p_ps1 = p1.enter_context(tc.tile_pool(name="ps1", bufs=6, space="PSUM"))
            p_ps2 = p1.enter_context(tc.tile_pool(name="ps2", bufs=2, space="PSUM"))

            xt = [p_x.tile([128, S], bf, tag="xt", name=f"xt{k}") for k in range(8)]
            wq_t = [p_w.tile([128, D], bf, tag="wq", name=f"wq{k}") for k in range(8)]
            wk_t = [p_w.tile([128, D], bf, tag="wk", name=f"wk{k}") for k in range(8)]
            wv_t = [p_w.tile([128, D], bf, tag="wv", name=f"wv{k}") for k in range(8)]
            q4 = S // 4
            for k in range(8):
                src_x = xT[k * 128 : (k + 1) * 128, :]
                for qq in range(4):
                    nc.sync.dma_start(
                        out=xt[k][:, qq * q4 : (qq + 1) * q4],
                        in_=src_x[:, qq * q4 : (qq + 1) * q4],
                    )
                src_w = Wq[k * 128 : (k + 1) * 128, :]
                for qq in range(4):
                    nc.sync.dma_start(
                        out=wq_t[k][:, qq * 256 : (qq + 1) * 256],
                        in_=src_w[:, qq * 256 : (qq + 1) * 256],
                    )
            nc.sync.dma_start(out=swp_sb, in_=SWP[:, :])
            nc.sync.dma_start(out=cos_sb, in_=COS2[:, :])
            nc.sync.dma_start(out=s_sb, in_=S2[:, :])
            for k in range(8):
                nc.sync.dma_start(out=wk_t[k], in_=Wk[k * 128 : (k + 1) * 128, :])
            for k in range(8):
                nc.sync.dma_start(out=wv_t[k], in_=Wv[k * 128 : (k + 1) * 128, :])
            for k in range(8):
                nc.sync.dma_start(out=wo_t[k], in_=Wo[k * 128 : (k + 1) * 128, :])
            nc.sync.dma_start(out=sel16, in_=aps["SEL"][:, :])

            for c in range(NCHUNK):
                nc.gpsimd.memset(krz[c][0][64:128, :], 0.0)
                nc.gpsimd.memset(krz[c][1][0:64, :], 0.0)

            # q/k in transposed layout + RoPE (k-outer for lhsT reuse)
            for which, w_t in (("q", wq_t), ("k", wk_t)):
                for c in range(NCHUNK):
                    raw = p_tmp.tile([128, S], bf, tag="raw")
                    pss = [
                        p_ps1.tile(
                            [128, 512], f32, tag="mm1", name=f"mm1_{which}{c}_{i}"
                        )
                        for i in range(3)
                    ]
                    for k in range(8):
                        for i, (off, wdt) in enumerate(ICH):
                            nc.tensor.matmul(
                                pss[i][:, :wdt],
                                w_t[k][:, c * 128 : (c + 1) * 128],
                                xt[k][:, off : off + wdt],
                                start=(k == 0),
                                stop=(k == 7),
                            )
                    for i, (off, wdt) in enumerate(ICH):
                        nc.scalar.copy(raw[:, off : off + wdt], pss[i][:, :wdt])
                    for off, wdt in ICH:
                        sw = p_ps2.tile([128, 512], f32, tag="swp")
                        nc.tensor.matmul(
                            sw[:, :wdt],
                            swp_sb,
                            raw[:, off : off + wdt],
                            start=True,
                            stop=True,
                        )
                        t1 = p_tmp.tile([128, 512], bf, tag="t1")
                        nc.vector.tensor_mul(
                            t1[:, :wdt],
                            raw[:, off : off + wdt],
                            cos_sb[:, off : off + wdt],
                        )
                        t2 = p_tmp.tile([128, 512], bf, tag="t2")
                        nc.vector.tensor_mul(
                            t2[:, :wdt], sw[:, :wdt], s_sb[:, off : off + wdt]
                        )
                        if which == "q":
                            nc.vector.tensor_add(
                                qr[c][:, off : off + wdt], t1[:, :wdt], t2[:, :wdt]
                            )
                        else:
                            nc.vector.tensor_add(
                                krz[c][0][0:64, off : off + wdt],
                                t1[0:64, :wdt],
                                t2[0:64, :wdt],
                            )
                            nc.vector.tensor_add(
                                krz[c][1][64:128, off : off + wdt],
                                t1[64:128, :wdt],
                                t2[64:128, :wdt],
                            )

            # v in natural layout, overlapped [v_h | 1] blocks (stride 65)
            for j in range(NJT):
                rows = 128 if j < 8 else TAIL
                vx3 = vx[j][:, :1040].rearrange("p (h d) -> p h d", d=65)
                nc.gpsimd.memset(vx[j][:, 1040:1104], 0.0)
                nc.gpsimd.memset(vx3[:rows, :, 64:65], 1.0)
                psv = [
                    p_ps1.tile([128, 512], f32, tag="mm1", name=f"mm1_v{j}_{i}")
                    for i in range(2)
                ]
                for k in range(8):
                    for ci in range(2):
                        nc.tensor.matmul(
                            psv[ci][:rows, :],
                            xt[k][:, j * 128 : j * 128 + rows],
                            wv_t[k][:, ci * 512 : (ci + 1) * 512],
                            start=(k == 0),
                            stop=(k == 7),
                        )
                for ci in range(2):
                    nc.scalar.copy(
                        vx3[:rows, ci * 8 : (ci + 1) * 8, 0:64],
                        psv[ci][:rows, :].rearrange("p (h d) -> p h d", h=8),
                    )

        # ================= phase 2: attention per head =================
        with ExitStack() as p2:
            p_pt = p2.enter_context(tc.tile_pool(name="pt", bufs=27))
            p_sm = p2.enter_context(tc.tile_pool(name="sm", bufs=1))
            p_st = p2.enter_context(tc.tile_pool(name="st", bufs=2, space="PSUM"))
            p_po = p2.enter_context(tc.tile_pool(name="po", bufs=2, space="PSUM"))

            cs_all = p_sm.tile([16, S], f32, tag="cs")
            for h in range(H):
                c, hb = divmod(h, 2)
                pts = []
                for j in range(NJT):
                    rows = 128 if j < 8 else TAIL
                    pt = p_pt.tile([128, S], bf, tag="pt", name=f"pt{h}_{j}")
                    pts.append(pt)
                    st = p_st.tile([128, S], f32, tag="st", name=f"st{h}_{j}")
                    for off, wdt in ICH:
                        nc.tensor.matmul(
                            st[:rows, off : off + wdt],
                            krz[c][hb][:, j * 128 : j * 128 + rows],
                            qr[c][:, off : off + wdt],
                            start=True,
                            stop=True,
                        )
                    nc.scalar.activation(
                        pt[:rows, :], st[:rows, :], Exp, scale=1.0 / np.sqrt(HD)
                    )
                cstage = p_sm.tile([65, S], f32, tag="cstage", name=f"cst{h}", bufs=2)
                for off, wdt in ICH:
                    ot = p_po.tile([128, 512], f32, tag="ot", name=f"ot{h}_{off}")
                    for j in range(NJT):
                        rows = 128 if j < 8 else TAIL
                        nc.tensor.matmul(
                            ot[:, :wdt],
                            vx[j][:rows, h * 65 : h * 65 + 128],
                            pts[j][:rows, off : off + wdt],
                            start=(j == 0),
                            stop=(j == NJT - 1),
                        )
                    nc.vector.tensor_mul(
                        otc[c][hb * 64 : hb * 64 + 64, off : off + wdt],
                        ot[0:64, :wdt],
                        ones64[:, off : off + wdt],
                    )
                    nc.vector.tensor_copy(
                        cstage[64:65, off : off + wdt], ot[64:65, :wdt]
                    )
                nc.sync.dma_start(out=cs_all[h : h + 1, :], in_=cstage[64:65, :])

            rcp_all = p_sm.tile([16, S], f32, tag="rcp")
            nc.vector.reciprocal_approx_fast(rcp_all, cs_all)
            for c in range(NCHUNK):
                for off, wdt in ICH:
                    psb = p_po.tile([128, 512], f32, tag="ot", name=f"psb{c}_{off}")
                    nc.tensor.matmul(
                        psb[:, :wdt],
                        sel16[:, c * 128 : (c + 1) * 128],
                        rcp_all[:, off : off + wdt],
                        start=True,
                        stop=True,
                    )
                    nc.vector.tensor_mul(
                        otc[c][:, off : off + wdt],
                        otc[c][:, off : off + wdt],
                        psb[:, :wdt],
                    )

        # ================= phase 3: output projection =================
        with ExitStack() as p3:
            p_y = p3.enter_context(tc.tile_pool(name="y", bufs=4))
            p_py = p3.enter_context(tc.tile_pool(name="py", bufs=4, space="PSUM"))

            for it in range(NJT):
                rows = 128 if it < 8 else TAIL
                yps = [
                    p_py.tile([128, 512], f32, tag="y", name=f"y{it}_{i}")
                    for i in range(2)
                ]
                for c in range(NCHUNK):
                    for ci in range(2):
                        nc.tensor.matmul(
                            yps[ci][:rows, :],
                            otc[c][:, it * 128 : it * 128 + rows],
                            wo_t[c][:, ci * 512 : (ci + 1) * 512],
                            start=(c == 0),
                            stop=(c == 7),
                        )
                for ci in range(2):
                    ysb = p_y.tile([128, 512], f32, tag="ysb")
                    nc.scalar.copy(ysb[:rows, :], yps[ci][:rows, :])
                    nc.sync.dma_start(
                        out=OUT[it * 128 : it * 128 + rows, ci * 512 : (ci + 1) * 512],
                        in_=ysb[:rows, :],
                    )


def _build():
    global _compiled
    if _compiled is not None:
        return _compiled
    import concourse.bass as bass  # noqa: F401
    import concourse.mybir as mybir
    import concourse.tile as tile
    from concourse import bacc

    nc = bacc.Bacc("TRN2", target_bir_lowering=False, debug=False)
    bf = mybir.dt.bfloat16
    f32 = mybir.dt.float32
    aps = {
        "xT": nc.dram_tensor("xT", [D, S], bf, kind="ExternalInput").ap(),
        "Wq": nc.dram_tensor("Wq", [D, H * HD], bf, kind="ExternalInput").ap(),
        "Wk": nc.dram_tensor("Wk", [D, H * HD], bf, kind="ExternalInput").ap(),
        "Wv": nc.dram_tensor("Wv", [D, H * HD], bf, kind="ExternalInput").ap(),
        "Wo": nc.dram_tensor("Wo", [H * HD, D], bf, kind="ExternalInput").ap(),
        "COS2": nc.dram_tensor("COS2", [128, S], bf, kind="ExternalInput").ap(),
        "S2": nc.dram_tensor("S2", [128, S], bf, kind="ExternalInput").ap(),
        "SWP": nc.dram_tensor("SWP", [128, 128], bf, kind="ExternalInput").ap(),
        "SEL": nc.dram_tensor("SEL", [16, 1024], f32, kind="ExternalInput").ap(),
        "out": nc.dram_tensor("out", [S, D], f32, kind="ExternalOutput").ap(),
    }
    with tile.TileContext(nc) as tc:
        _build_body(nc, tc, tile, mybir, aps)
    nc.compile()
    _compiled = nc
    return nc


def _install_trace_shim():
    """The agent image's antenv lacks axon_hooks, so run_bass_kernel_spmd's
    trace path can't find the NTFF profile hook trn_boot would have set.
    Recreate the module and install the ctypes hook; skip the S3 artifact
    upload (no creds needed for local timing)."""
    import sys
    import types

    if "antenv.axon_hooks" not in sys.modules:
        import antenv  # noqa: F401

        mod = types.ModuleType("antenv.axon_hooks")
        mod._hook = None

        def set_axon_ntff_profile_hook(h):
            mod._hook = h

        def get_axon_ntff_profile_hook():
            return mod._hook

        mod.set_axon_ntff_profile_hook = set_axon_ntff_profile_hook
        mod.get_axon_ntff_profile_hook = get_axon_ntff_profile_hook
        sys.modules["antenv.axon_hooks"] = mod

    import antenv.axon_hooks as ah

    if ah.get_axon_ntff_profile_hook() is None:
        from trn_agent_boot.trn_boot import _ntff_profile_via_ctypes

        ah.set_axon_ntff_profile_hook(
            _ntff_profile_via_ctypes("/opt/axon/libaxon_pjrt.so")
        )

    import concourse.bass_utils as bu

    bu.upload_artifacts = lambda tmpdir: f"local://{tmpdir}"


def run(inputs, trace=False):
    """Returns (output (8,1040,1024) f32, exec_time_ns or None)."""
    if trace:
        _install_trace_shim()
    from concourse.bass_utils import run_bass_kernel_spmd

    nc = _build()
    x = np.asarray(inputs["x"], np.float32)
    wq = np.asarray(inputs["Wq"], np.float32).reshape(D, H * HD).astype(BF16)
    wk = np.asarray(inputs["Wk"], np.float32).reshape(D, H * HD).astype(BF16)
    wv = np.asarray(inputs["Wv"], np.float32).reshape(D, H * HD).astype(BF16)
    wo = np.asarray(inputs["Wo"], np.float32).reshape(H * HD, D).astype(BF16)
    cos2, s2 = _rope_tables()
    swp = _swap_matrix()
    sel = np.zeros((16, 1024), np.float32)
    for c in range(8):
        for hb in range(2):
            sel[2 * c + hb, c * 128 + hb * 64 : c * 128 + hb * 64 + 64] = 1.0
    shared = {
        "Wq": wq, "Wk": wk, "Wv": wv, "Wo": wo,
        "COS2": cos2, "S2": s2, "SWP": swp, "SEL": sel,
    }
    in_maps = [
        dict(shared, xT=np.ascontiguousarray(x[b].T).astype(BF16)) for b in range(B)
    ]
    res = run_bass_kernel_spmd(nc, in_maps, core_ids=list(range(B)), trace=trace)
    out = np.stack([np.asarray(r["out"], np.float32) for r in res.results], axis=0)
    return out, res.exec_time_ns


def kernel(x, Wq, Wk, Wv, Wo):
    out, _ = run({"x": x, "Wq": Wq, "Wk": Wk, "Wv": Wv, "Wo": Wo})
    return out
